# revision 18
# baseline (speedup 1.0000x reference)
"""Kernel-target-alignment loss on 8 TRN2 NeuronCores.

Math: Xs = X*sqrt(params); d2_ij = ||Xs_i - Xs_j||^2; K = exp(-d2) (diag == 1);
kta = sum(K*tt^T) / (N*sqrt(sum(K*K)));  return -kta.

v2 strategy — symmetric-triangle, cyclic row sharding, ACT-bound:
  K is symmetric, so only the (block-)upper triangle is computed.  Global row
  blocks (128 rows each) are assigned cyclically: core c owns rb_g = 8k + c,
  k = 0..7.  For local block k, column tiles ct >= k are kept (36 of 64 tiles
  per core, perfectly balanced): ct == k is the diagonal-crossing tile
  (weight 1 — across all rbs these tile the diagonal superblocks exactly),
  ct > k is strictly above (weight 2).

  Per tile [128, 1024]:
    PE  : A = 2*Xs@Xs^T - sq_j via one augmented fp32r matmul
          (lhsT = [2p*x ; 1], rhs = [x ; -sq]); fp32r runs at 1 cycle/row
          (plain fp32 is 4x slower).
    ACT : E = bf16 exp(A + bias) with bias = -sq_i  (the only exp pass;
          ACT is the bottleneck engine at ~41us/core).
    DVE : E*E with accum -> per-(tile,partition) partial of sum(K^2).
    PE  : q2[ct] += tw^T @ E  (column sums weighted by t_i, 2*t_i for
          above-diagonal tiles) accumulated in PSUM across the column tile's
          row blocks; emitted one tile late so PE never waits on ACT.

  -sq and 2p*x are precomputed on the host (O(N*D) fp32 prep, bit-compatible
  with the device arithmetic): A_ii = 2*(G_ii - sq_i) stays ~1e-4, so
  bf16(exp(A_ii)) == 1.0, matching the reference's unit diagonal.

  Host combine: s1 = sum_slots w * s1acc; s2 = sum_ct q2[ct] . t[ct];
  loss = -s2 / (N * sqrt(s1)).
"""

import numpy as np

import concourse.bass as bass
import concourse.bacc as bacc
import concourse.tile as tile
import concourse.mybir as mybir
from concourse.bass_utils import run_bass_kernel_spmd

N = 8192
D = 64
NCORES = 8
RPC = N // NCORES          # 1024 rows per core
NRB = RPC // 128           # 8 row blocks of 128 rows
CW = 1024                  # column tile width (2 PSUM banks fp32)
NCT = N // CW              # 8 column tiles
NSLOT = (NRB * (NRB + 1)) // 2  # 36 kept tiles per core

F32 = mybir.dt.float32
F32R = mybir.dt.float32r
BF16 = mybir.dt.bfloat16


def _ap(tensor, ap, offset=0):
    return bass.AP(tensor=tensor, offset=offset, ap=ap)


def build_kernel(variant="v2", reps=1):
    nc = bacc.Bacc("TRN2", target_bir_lowering=False)

    mm_r = variant != "v2f32"  # fp32r main matmuls unless disabled
    MDT = F32R if mm_r else F32

    # xt/l2p/nsq hold fp32r-rounded data (host pre-rounds); l2p's last row is
    # the constant 1.0 for the augmented -sq column term.
    xt_d = nc.dram_tensor("xt", [D, N], MDT, kind="ExternalInput")
    l2p_d = nc.dram_tensor("l2p", [D + 1, RPC], MDT, kind="ExternalInput")
    nsq_d = nc.dram_tensor("nsq", [N], MDT, kind="ExternalInput")
    nsqw_d = nc.dram_tensor("nsqw", [128, NRB], F32, kind="ExternalInput")
    twf_d = nc.dram_tensor("twf", [128, NRB], F32, kind="ExternalInput")
    s1o_d = nc.dram_tensor("s1o", [128, NSLOT], F32, kind="ExternalOutput")
    s2o_d = nc.dram_tensor("s2o", [1, N], F32, kind="ExternalOutput")

    with tile.TileContext(nc) as tc:
      for _rep in range(reps):
        with (
            tc.tile_pool(name="const", bufs=1) as cpool,
            tc.tile_pool(name="etile", bufs=4) as epool,
            tc.tile_pool(name="scratch", bufs=2) as spool,
            tc.tile_pool(name="mmpsum", bufs=2, space="PSUM") as mpool,
            tc.tile_pool(name="q2psum", bufs=2, space="PSUM") as qpool,
        ):
            # ---- persistent SBUF tensors -------------------------------------
            # R is split per column tile so tile (ct=0, k=0) can start as soon
            # as the first x^T chunk lands, overlapping the rest of the load.
            Rc = [
                cpool.tile([D + 1, CW], MDT, tag=f"R{ct}", name=f"R{ct}")
                for ct in range(NCT)
            ]                                              # [x^T ; -sq] chunks
            L = cpool.tile([D + 1, RPC], MDT, tag="L")     # [2p*x^T ; ones]
            rsqn = cpool.tile([128, NRB], F32, tag="rsqn")  # -sq rows (bias)
            twf = cpool.tile([128, NRB], F32, tag="twf")
            tw1 = cpool.tile([128, NRB], BF16, tag="tw1")   # t rows
            tw2 = cpool.tile([128, NRB], BF16, tag="tw2")   # 2*t rows
            s1acc = cpool.tile([128, NSLOT], F32, tag="s1acc")
            q2s = cpool.tile([1, N], F32, tag="q2s")        # staged q2 sums

            # ---- setup: pure DMAs + two tiny casts ---------------------------
            # scalar (ACT hwdge) ring: small control loads first
            nc.scalar.dma_start(out=rsqn[:, :], in_=nsqw_d[:, :])
            nc.scalar.dma_start(out=twf[:, :], in_=twf_d[:, :])
            nc.scalar.dma_start(out=L[:, :], in_=l2p_d[:, :])
            for ct in range(NCT):
                nc.scalar.dma_start(
                    out=Rc[ct][D : D + 1, :],
                    in_=_ap(nsq_d, [[0, 1], [1, CW]], offset=ct * CW),
                )
            # sync (SP hwdge) ring: bulk x^T chunks in ct order
            for ct in range(NCT):
                sl = slice(ct * CW, (ct + 1) * CW)
                nc.sync.dma_start(out=Rc[ct][0:D, :], in_=xt_d[:, sl])
            nc.vector.tensor_copy(out=tw1[:, :], in_=twf[:, :])
            nc.vector.tensor_scalar_mul(tw2[:, :], twf[:, :], 2.0)

            # ---- main loop: ct-major over the kept triangle ------------------
            # pending E*t matmuls are emitted one tile late so the PE never
            # stalls waiting on ACT's exp of the current tile.
            pending = None

            def flush_pending():
                nonlocal pending
                if pending is None:
                    return
                q2, k, ct, E = pending
                tw = tw1 if k == ct else tw2
                for h in range(2):
                    hs = slice(h * 512, (h + 1) * 512)
                    nc.tensor.matmul(
                        q2[0:1, hs],
                        tw[:, k : k + 1],
                        E[:, hs],
                        start=(k == 0),
                        stop=(k == ct),
                    )
                if k == ct:  # group closed -> stage this column tile's sums
                    nc.vector.tensor_copy(
                        out=q2s[0:1, ct * CW : (ct + 1) * CW], in_=q2[0:1, :]
                    )
                pending = None

            slot = 0
            for ct in range(NCT):
                csl = slice(ct * CW, (ct + 1) * CW)
                q2 = qpool.tile([1, CW], F32, tag="q2")
                for k in range(ct + 1):
                    lhsT = L[:, k * 128 : (k + 1) * 128]
                    mm = mpool.tile([128, CW], F32, tag="mm")
                    for j in range(CW // 512):
                        sl = slice(j * 512, (j + 1) * 512)
                        nc.tensor.matmul(
                            mm[:, j * 512 : (j + 1) * 512],
                            lhsT,
                            Rc[ct][:, sl],
                            start=True,
                            stop=True,
                        )
                    flush_pending()
                    E = epool.tile([128, CW], BF16, tag="E")
                    nc.scalar.activation(
                        out=E[:, :], in_=mm[:, :],
                        func=mybir.ActivationFunctionType.Exp,
                        bias=rsqn[:, k : k + 1], scale=1.0,
                    )
                    sc1 = spool.tile([128, CW], BF16, tag="sc1")
                    nc.vector.scalar_tensor_tensor(
                        out=sc1[:, :], in0=E[:, :], scalar=1.0, in1=E[:, :],
                        op0=mybir.AluOpType.mult, op1=mybir.AluOpType.mult,
                        accum_out=s1acc[:, slot : slot + 1],
                    )
                    pending = (q2, k, ct, E)
                    slot += 1
            flush_pending()

            nc.sync.dma_start(out=s1o_d[:, :], in_=s1acc[:, :])
            nc.sync.dma_start(out=s2o_d[0:1, :], in_=q2s[0:1, :])

    nc.compile()
    return nc


_NC_CACHE = None


def _slot_weights():
    w = np.empty(NSLOT, dtype=np.float64)
    s = 0
    for ct in range(NCT):
        for k in range(ct + 1):
            w[s] = 1.0 if k == ct else 2.0
            s += 1
    return w


_W = _slot_weights()


def to_fp32r(a):
    """Round fp32 to fp32r (E8M11: low 12 mantissa bits zero), RNE."""
    u = np.ascontiguousarray(a, dtype=np.float32).view(np.uint32)
    low = u & np.uint32(0xFFF)
    trunc = u & np.uint32(0xFFFFF000)
    half = np.uint32(0x800)
    odd = (trunc >> np.uint32(12)) & np.uint32(1)
    round_up = (low > half) | ((low == half) & (odd == 1))
    out = trunc + (round_up.astype(np.uint32) << np.uint32(12))
    return out.view(np.float32).reshape(np.shape(a))


def make_in_maps(X, target, params):
    X = np.ascontiguousarray(X, dtype=np.float32)
    target = np.ascontiguousarray(target, dtype=np.float32)
    params = np.ascontiguousarray(params, dtype=np.float32)
    xt_r = to_fp32r(np.ascontiguousarray(X.T))          # [D, N] fp32r
    p2 = (2.0 * params).astype(np.float32)
    # the PE computes M_ij = sum_d l2p_r[d,i] * xt_r[d,j]; the diagonal must
    # cancel exactly: augmented row adds u_j = fp32r(-M_jj/2), ACT bias adds
    # b_i = -M_ii - u_i (full fp32), so A_ii = M_ii + u_i + b_i = 0.
    l2p_all = to_fp32r(p2[:, None] * xt_r)              # [D, N] fp32r
    M_diag = np.einsum(
        "dn,dn->n", l2p_all.astype(np.float64), xt_r.astype(np.float64)
    )                                                   # [N] exact
    u = to_fp32r((-M_diag / 2.0).astype(np.float32))    # [N] fp32r
    b = (-M_diag - u.astype(np.float64)).astype(np.float32)  # [N] fp32 bias
    ones = np.ones((1, RPC), dtype=np.float32)
    maps = []
    for c in range(NCORES):
        rows = (
            np.arange(NRB)[:, None] * RPC + 128 * c + np.arange(128)[None, :]
        ).ravel()                                       # local rows, k-major
        maps.append(
            {
                "xt": xt_r,
                "l2p": np.concatenate([l2p_all[:, rows], ones], axis=0),
                "nsq": u,
                "nsqw": np.ascontiguousarray(b[rows].reshape(NRB, 128).T),
                "twf": np.ascontiguousarray(target[rows].reshape(NRB, 128).T),
            }
        )
    return maps


def combine(results, target):
    t64 = target.astype(np.float64)
    s1 = 0.0
    s2 = 0.0
    for c in range(NCORES):
        s1o = results[c]["s1o"].astype(np.float64)      # [128, NSLOT]
        s2o = results[c]["s2o"].astype(np.float64)      # [1, N]
        s1 += float(s1o.sum(axis=0) @ _W)
        s2 += float(s2o.ravel() @ t64)
    val = -s2 / (N * np.sqrt(s1))
    return np.array(val, dtype=np.float32)


def kernel(X, target, params):
    global _NC_CACHE
    X = np.ascontiguousarray(X, dtype=np.float32)
    target = np.ascontiguousarray(target, dtype=np.float32)
    params = np.ascontiguousarray(params, dtype=np.float32)

    in_maps = make_in_maps(X, target, params)

    if _NC_CACHE is None:
        _NC_CACHE = build_kernel()
    res = run_bass_kernel_spmd(_NC_CACHE, in_maps, core_ids=list(range(NCORES)))
    return combine(res.results, target)


# revision 26
# speedup vs baseline: 1.0296x; 1.0296x over previous
"""Kernel-target-alignment loss on 8 TRN2 NeuronCores.

Math: Xs = X*sqrt(params); d2_ij = ||Xs_i - Xs_j||^2; K = exp(-d2) (diag == 1);
kta = sum(K*tt^T) / (N*sqrt(sum(K*K)));  return -kta.

v2 strategy — symmetric-triangle, cyclic row sharding, ACT-bound:
  K is symmetric, so only the (block-)upper triangle is computed.  Global row
  blocks (128 rows each) are assigned cyclically: core c owns rb_g = 8k + c,
  k = 0..7.  For local block k, column tiles ct >= k are kept (36 of 64 tiles
  per core, perfectly balanced): ct == k is the diagonal-crossing tile
  (weight 1 — across all rbs these tile the diagonal superblocks exactly),
  ct > k is strictly above (weight 2).

  Per tile [128, 1024]:
    PE  : A = 2*Xs@Xs^T - sq_j via one augmented fp32r matmul
          (lhsT = [2p*x ; 1], rhs = [x ; -sq]); fp32r runs at 1 cycle/row
          (plain fp32 is 4x slower).
    ACT : E = bf16 exp(A + bias) with bias = -sq_i  (the only exp pass;
          ACT is the bottleneck engine at ~41us/core).
    DVE : E*E with accum -> per-(tile,partition) partial of sum(K^2).
    PE  : q2[ct] += tw^T @ E  (column sums weighted by t_i, 2*t_i for
          above-diagonal tiles) accumulated in PSUM across the column tile's
          row blocks; emitted one tile late so PE never waits on ACT.

  -sq and 2p*x are precomputed on the host (O(N*D) fp32 prep, bit-compatible
  with the device arithmetic): A_ii = 2*(G_ii - sq_i) stays ~1e-4, so
  bf16(exp(A_ii)) == 1.0, matching the reference's unit diagonal.

  Host combine: s1 = sum_slots w * s1acc; s2 = sum_ct q2[ct] . t[ct];
  loss = -s2 / (N * sqrt(s1)).
"""

import numpy as np

import concourse.bass as bass
import concourse.bacc as bacc
import concourse.tile as tile
import concourse.mybir as mybir
from concourse.bass_utils import run_bass_kernel_spmd

N = 8192
D = 64
NCORES = 8
RPC = N // NCORES          # 1024 rows per core
NRB = RPC // 128           # 8 row blocks of 128 rows
CW = 1024                  # column tile width (2 PSUM banks fp32)
NCT = N // CW              # 8 column tiles
NSLOT = (NRB * (NRB + 1)) // 2  # 36 kept tiles per core

F32 = mybir.dt.float32
F32R = mybir.dt.float32r
BF16 = mybir.dt.bfloat16


def _ap(tensor, ap, offset=0):
    return bass.AP(tensor=tensor, offset=offset, ap=ap)


def _assign():
    """Greedy per-tile engine assignment balancing measured per-pass costs.

    Returns per-slot (e2_engine, et_engine) with e2 in {act, dve} (sum K^2
    partial via 2nd exp(2A)+accum on ACT, or E*E stt on DVE) and et in
    {pe, dve} (t-weighted column sums via PE matmul into PSUM, or E*t row
    sums via DVE stt).  Costs in us per [128,1024] tile, measured on HW.
    """
    act = 0.5    # table load
    dve = 0.5    # casts etc.
    pe = 0.0
    out = []
    for ct in range(NCT):
        pe_in_ct = False
        for k in range(ct + 1):
            # fixed per-tile work accrues incrementally so the greedy
            # interleaves assignments across time, not just in total.
            act += 1.111  # exp
            pe += 1.061   # 2 main matmuls
            e2 = "act" if act + 1.111 <= dve + 1.199 else "dve"
            if e2 == "act":
                act += 1.111
            else:
                dve += 1.199
            et = "pe" if pe + 0.975 <= dve + 1.199 else "dve"
            if et == "pe":
                pe += 0.975
                pe_in_ct = True
            else:
                dve += 1.199
            out.append((e2, et))
        if pe_in_ct:
            dve += 1.19  # PSUM->SBUF staging copy of this ct's q2
    return out


_ASSIGN = _assign()


def build_kernel(variant="v2", reps=1):
    nc = bacc.Bacc("TRN2", target_bir_lowering=False)

    mm_r = variant != "v2f32"  # fp32r main matmuls unless disabled
    MDT = F32R if mm_r else F32

    # xt/l2p/nsq hold fp32r-rounded data (host pre-rounds); l2p's last row is
    # the constant 1.0 for the augmented -sq column term.
    xt_d = nc.dram_tensor("xt", [D, N], MDT, kind="ExternalInput")
    l2p_d = nc.dram_tensor("l2p", [D + 1, RPC], MDT, kind="ExternalInput")
    nsq_d = nc.dram_tensor("nsq", [N], MDT, kind="ExternalInput")
    nsqw_d = nc.dram_tensor("nsqw", [128, NRB], F32, kind="ExternalInput")
    twf_d = nc.dram_tensor("twf", [128, NRB], F32, kind="ExternalInput")
    tcb_d = nc.dram_tensor("tcb", [128, N], BF16, kind="ExternalInput")
    s1o_d = nc.dram_tensor("s1o", [128, NSLOT], F32, kind="ExternalOutput")
    s2o_d = nc.dram_tensor("s2o", [1, N], F32, kind="ExternalOutput")
    s2r_d = nc.dram_tensor("s2r", [128, NSLOT], F32, kind="ExternalOutput")

    with tile.TileContext(nc) as tc:
      for _rep in range(reps):
        with (
            tc.tile_pool(name="const", bufs=1) as cpool,
            tc.tile_pool(name="etile", bufs=4) as epool,
            tc.tile_pool(name="scratch", bufs=2) as spool,
            tc.tile_pool(name="mmpsum", bufs=2, space="PSUM") as mpool,
            tc.tile_pool(name="q2psum", bufs=2, space="PSUM") as qpool,
        ):
            # ---- persistent SBUF tensors -------------------------------------
            # R is split per column tile so tile (ct=0, k=0) can start as soon
            # as the first x^T chunk lands, overlapping the rest of the load.
            Rc = [
                cpool.tile([D + 1, CW], MDT, tag=f"R{ct}", name=f"R{ct}")
                for ct in range(NCT)
            ]                                              # [x^T ; -sq] chunks
            L = cpool.tile([D + 1, RPC], MDT, tag="L")     # [2p*x^T ; ones]
            rsqn = cpool.tile([128, NRB], F32, tag="rsqn")  # -sq rows (bias)
            rsqn2 = cpool.tile([128, NRB], F32, tag="rsqn2")  # 2x bias
            twf = cpool.tile([128, NRB], F32, tag="twf")
            tw1 = cpool.tile([128, NRB], BF16, tag="tw1")   # t rows
            tw2 = cpool.tile([128, NRB], BF16, tag="tw2")   # 2*t rows
            tcb = cpool.tile([128, N], BF16, tag="tcb")     # t broadcast
            s1acc = cpool.tile([128, NSLOT], F32, tag="s1acc")
            s2acc = cpool.tile([128, NSLOT], F32, tag="s2acc")
            q2s = cpool.tile([1, N], F32, tag="q2s")        # staged q2 sums

            # ---- setup: pure DMAs + two tiny casts ---------------------------
            # scalar (ACT hwdge) ring: small control loads first
            nc.scalar.dma_start(out=rsqn[:, :], in_=nsqw_d[:, :])
            nc.scalar.dma_start(out=twf[:, :], in_=twf_d[:, :])
            nc.scalar.dma_start(out=L[:, :], in_=l2p_d[:, :])
            for ct in range(NCT):
                nc.scalar.dma_start(
                    out=Rc[ct][D : D + 1, :],
                    in_=_ap(nsq_d, [[0, 1], [1, CW]], offset=ct * CW),
                )
            # sync (SP hwdge) ring: bulk x^T and t-broadcast chunks in ct order
            for ct in range(NCT):
                sl = slice(ct * CW, (ct + 1) * CW)
                nc.sync.dma_start(out=Rc[ct][0:D, :], in_=xt_d[:, sl])
                nc.sync.dma_start(out=tcb[:, sl], in_=tcb_d[:, sl])
            nc.vector.memset(s2acc[:, :], 0.0)
            nc.vector.tensor_copy(out=tw1[:, :], in_=twf[:, :])
            nc.vector.tensor_scalar_mul(tw2[:, :], twf[:, :], 2.0)
            nc.vector.tensor_scalar_mul(rsqn2[:, :], rsqn[:, :], 2.0)

            # ---- main loop: ct-major over the kept triangle ------------------
            # pending E*t matmuls are emitted one tile late so the PE never
            # stalls waiting on ACT's exp of the current tile.
            pending = None

            def flush_pending():
                nonlocal pending
                if pending is None:
                    return
                q2, k, ct, E, first, last = pending
                tw = tw1 if k == ct else tw2
                for h in range(2):
                    hs = slice(h * 512, (h + 1) * 512)
                    nc.tensor.matmul(
                        q2[0:1, hs],
                        tw[:, k : k + 1],
                        E[:, hs],
                        start=first,
                        stop=last,
                    )
                if last:  # group closed -> stage this column tile's sums
                    nc.vector.tensor_copy(
                        out=q2s[0:1, ct * CW : (ct + 1) * CW], in_=q2[0:1, :]
                    )
                pending = None

            slot = 0
            pe_cts = []
            for ct in range(NCT):
                base = ct * (ct + 1) // 2
                ks_pe = [
                    k for k in range(ct + 1) if _ASSIGN[base + k][1] == "pe"
                ]
                if ks_pe:
                    pe_cts.append(ct)
                    q2 = qpool.tile([1, CW], F32, tag="q2")
                for k in range(ct + 1):
                    e2_eng, et_eng = _ASSIGN[base + k]
                    lhsT = L[:, k * 128 : (k + 1) * 128]
                    mm = mpool.tile([128, CW], F32, tag="mm")
                    for j in range(CW // 512):
                        sl = slice(j * 512, (j + 1) * 512)
                        nc.tensor.matmul(
                            mm[:, j * 512 : (j + 1) * 512],
                            lhsT,
                            Rc[ct][:, sl],
                            start=True,
                            stop=True,
                        )
                    flush_pending()
                    E = epool.tile([128, CW], BF16, tag="E")
                    nc.scalar.activation(
                        out=E[:, :], in_=mm[:, :],
                        func=mybir.ActivationFunctionType.Exp,
                        bias=rsqn[:, k : k + 1], scale=1.0,
                    )
                    if e2_eng == "act":
                        # sum K^2 partial straight off PSUM: exp(2A) + accum
                        E2 = epool.tile([128, CW], BF16, tag="E2")
                        nc.scalar.activation(
                            out=E2[:, :], in_=mm[:, :],
                            func=mybir.ActivationFunctionType.Exp,
                            bias=rsqn2[:, k : k + 1], scale=2.0,
                            accum_out=s1acc[:, slot : slot + 1],
                        )
                    else:
                        sc1 = spool.tile([128, CW], BF16, tag="sc1")
                        nc.vector.scalar_tensor_tensor(
                            out=sc1[:, :], in0=E[:, :], scalar=1.0, in1=E[:, :],
                            op0=mybir.AluOpType.mult, op1=mybir.AluOpType.mult,
                            accum_out=s1acc[:, slot : slot + 1],
                        )
                    if et_eng == "pe":
                        pending = (
                            q2, k, ct, E, k == ks_pe[0], k == ks_pe[-1]
                        )
                    else:
                        sc2 = spool.tile([128, CW], BF16, tag="sc2")
                        nc.vector.scalar_tensor_tensor(
                            out=sc2[:, :], in0=E[:, :], scalar=1.0,
                            in1=tcb[:, ct * CW : (ct + 1) * CW],
                            op0=mybir.AluOpType.mult, op1=mybir.AluOpType.mult,
                            accum_out=s2acc[:, slot : slot + 1],
                        )
                    slot += 1
            flush_pending()

            nc.sync.dma_start(out=s1o_d[:, :], in_=s1acc[:, :])
            nc.sync.dma_start(out=s2r_d[:, :], in_=s2acc[:, :])
            for ct in pe_cts:
                sl = slice(ct * CW, (ct + 1) * CW)
                nc.scalar.dma_start(out=s2o_d[0:1, sl], in_=q2s[0:1, sl])

    nc.compile()
    return nc


_NC_CACHE = None


def _slot_weights():
    w = np.empty(NSLOT, dtype=np.float64)
    s = 0
    for ct in range(NCT):
        for k in range(ct + 1):
            w[s] = 1.0 if k == ct else 2.0
            s += 1
    return w


_W = _slot_weights()


def to_fp32r(a):
    """Round fp32 to fp32r (E8M11: low 12 mantissa bits zero), RNE."""
    u = np.ascontiguousarray(a, dtype=np.float32).view(np.uint32)
    low = u & np.uint32(0xFFF)
    trunc = u & np.uint32(0xFFFFF000)
    half = np.uint32(0x800)
    odd = (trunc >> np.uint32(12)) & np.uint32(1)
    round_up = (low > half) | ((low == half) & (odd == 1))
    out = trunc + (round_up.astype(np.uint32) << np.uint32(12))
    return out.view(np.float32).reshape(np.shape(a))


def make_in_maps(X, target, params):
    X = np.ascontiguousarray(X, dtype=np.float32)
    target = np.ascontiguousarray(target, dtype=np.float32)
    params = np.ascontiguousarray(params, dtype=np.float32)
    xt_r = to_fp32r(np.ascontiguousarray(X.T))          # [D, N] fp32r
    p2 = (2.0 * params).astype(np.float32)
    # the PE computes M_ij = sum_d l2p_r[d,i] * xt_r[d,j]; the diagonal must
    # cancel exactly: augmented row adds u_j = fp32r(-M_jj/2), ACT bias adds
    # b_i = -M_ii - u_i (full fp32), so A_ii = M_ii + u_i + b_i = 0.
    l2p_all = to_fp32r(p2[:, None] * xt_r)              # [D, N] fp32r
    M_diag = np.einsum(
        "dn,dn->n", l2p_all.astype(np.float64), xt_r.astype(np.float64)
    )                                                   # [N] exact
    u = to_fp32r((-M_diag / 2.0).astype(np.float32))    # [N] fp32r
    b = (-M_diag - u.astype(np.float64)).astype(np.float32)  # [N] fp32 bias
    ones = np.ones((1, RPC), dtype=np.float32)
    bf16 = mybir.dt.np(BF16)
    tcb = np.ascontiguousarray(
        np.broadcast_to(target.astype(bf16), (128, N))
    )                                                   # [128, N] bf16
    maps = []
    for c in range(NCORES):
        rows = (
            np.arange(NRB)[:, None] * RPC + 128 * c + np.arange(128)[None, :]
        ).ravel()                                       # local rows, k-major
        maps.append(
            {
                "xt": xt_r,
                "l2p": np.concatenate([l2p_all[:, rows], ones], axis=0),
                "nsq": u,
                "nsqw": np.ascontiguousarray(b[rows].reshape(NRB, 128).T),
                "twf": np.ascontiguousarray(target[rows].reshape(NRB, 128).T),
                "tcb": tcb,
            }
        )
    return maps


def combine(results, target):
    t64 = target.astype(np.float64)
    dve_slots = np.array([i for i, a in enumerate(_ASSIGN) if a[1] == "dve"])
    slot_k = np.concatenate(
        [np.arange(ct + 1) for ct in range(NCT)]
    )                                                   # k per ct-major slot
    s1 = 0.0
    s2 = 0.0
    for c in range(NCORES):
        s1o = results[c]["s1o"].astype(np.float64)      # [128, NSLOT]
        s2o = results[c]["s2o"].astype(np.float64)      # [1, N] (PE-et cts)
        s2r = results[c]["s2r"].astype(np.float64)      # [128, NSLOT]
        s1 += float(s1o.sum(axis=0) @ _W)
        s2 += float(s2o.ravel() @ t64)
        if len(dve_slots):
            rows = (
                np.arange(NRB)[:, None] * RPC
                + 128 * c
                + np.arange(128)[None, :]
            ).ravel()
            t_loc = t64[rows].reshape(NRB, 128).T       # [128, NRB]
            ks = slot_k[dve_slots]
            # row sums * t_i, weighted by tile multiplicity
            s2 += float(
                np.sum(_W[dve_slots] * (t_loc[:, ks] * s2r[:, dve_slots]))
            )
    val = -s2 / (N * np.sqrt(s1))
    return np.array(val, dtype=np.float32)


def kernel(X, target, params):
    global _NC_CACHE
    X = np.ascontiguousarray(X, dtype=np.float32)
    target = np.ascontiguousarray(target, dtype=np.float32)
    params = np.ascontiguousarray(params, dtype=np.float32)

    in_maps = make_in_maps(X, target, params)

    if _NC_CACHE is None:
        _NC_CACHE = build_kernel()
    res = run_bass_kernel_spmd(_NC_CACHE, in_maps, core_ids=list(range(NCORES)))
    return combine(res.results, target)
